# revision 1
# baseline (speedup 1.0000x reference)
"""Trainium2 Bass kernel for AttnAugmentation2d (8 cores, batch-parallel).

Contract: kernel(**inputs) takes FULL inputs
  x [8, 768, 32, 32] f32, rel_w [63, 32] f32, rel_h [63, 32] f32
and returns the FULL output [8, 256, 32, 32] f32.

Sharding: data-parallel over batch — core b computes batch element b.

Per-core computation (channels-on-partitions layout, l = x*32 + y):
  S^T[m, l] = sum_d k[d,m] q[d,l] + Wc[y'(m), l] + Hc[x'(m), l]
  computed as ONE matmul with K=96: k rows plus 0/1 selector rows that
  broadcast the compact relative-position tables Wc/Hc [32, 1024].
  Wc/Hc come from block-diagonal matmuls (4 heads at once, K=128)
  against host-built kron(I4, rel_w.T) shifted tables, per y (resp. x).
  P^T = exp(S^T) (no max-subtraction; logits are O(6), fp32-exp-safe).
  out^T[d, l] = sum_m vT[m, d] P^T[m, l]; an appended ones column in vT
  (host-baked, DMA-transposed HBM->SBUF) yields the softmax denominator;
  the denominator is broadcast over partitions on GPSIMD, reciprocated
  and multiplied in on the vector engine.
Matmul operands are bf16 (1 PE cycle/row); accumulation is fp32 PSUM.
"""

import numpy as np
import ml_dtypes

import concourse.bacc as bacc
import concourse.mybir as mybir
from concourse import tile
from concourse.bass_utils import run_bass_kernel_spmd

F32 = mybir.dt.float32
BF16 = mybir.dt.bfloat16
AF = mybir.ActivationFunctionType
BF = np.dtype(ml_dtypes.bfloat16)

NH = 8
HW = 1024
SCALE = 32.0 ** -0.5

_CACHE = {}


def _host_prep_consts(rel_w: np.ndarray, rel_h: np.ndarray):
    # 32 shifted block-diagonal weight tables, one contiguous [128,128]
    # slab per y (walrus requires 1 free dim on matmul weight APs)
    i4 = np.eye(4, dtype=np.float32)
    rwT = np.asarray(rel_w, dtype=np.float32).T
    rhT = np.asarray(rel_h, dtype=np.float32).T
    relw4 = np.kron(i4, rwT)  # [128, 252]
    relh4 = np.kron(i4, rhT)
    m = np.arange(HW)
    wsel = (m[None, :] % 32 == np.arange(32)[:, None]).astype(np.float32)
    hsel = (m[None, :] // 32 == np.arange(32)[:, None]).astype(np.float32)
    sel = np.concatenate([wsel, hsel], axis=0)  # [64, 1024]
    consts = dict(
        relw4=relw4.astype(BF),
        relh4=relh4.astype(BF),
    )
    return consts, sel.astype(BF)


def _host_prep_core(x_b: np.ndarray, sel_bf: np.ndarray):
    xf = np.ascontiguousarray(np.asarray(x_b, np.float32).reshape(768, HW))
    q4 = (xf[0:256] * SCALE).astype(BF)  # scaled in fp32, rounded once
    lhs = np.empty((NH, 96, HW), dtype=BF)
    for h in range(NH):
        lhs[h, 0:32] = xf[256 + 32 * h : 288 + 32 * h].astype(BF)
        lhs[h, 32:96] = sel_bf
    v = np.zeros((NH, 48, HW), dtype=BF)
    for h in range(NH):
        v[h, 0:32] = xf[512 + 32 * h : 544 + 32 * h].astype(BF)
        v[h, 32] = 1.0
    return dict(q4=q4, lhs=lhs, v=v)


def build_nc(niters: int = 1, num_devices: int = 8):
    nc = bacc.Bacc(None, target_bir_lowering=False, debug=False,
                   num_devices=num_devices)

    q4_d = nc.dram_tensor("q4", [256, HW], BF16, kind="ExternalInput").ap()
    lhs_d = nc.dram_tensor("lhs", [NH, 96, HW], BF16, kind="ExternalInput").ap()
    v_d = nc.dram_tensor("v", [NH, 48, HW], BF16, kind="ExternalInput").ap()
    relw4_d = nc.dram_tensor("relw4", [128, 252], BF16, kind="ExternalInput").ap()
    relh4_d = nc.dram_tensor("relh4", [128, 252], BF16, kind="ExternalInput").ap()
    out_d = nc.dram_tensor("out", [256, HW], F32, kind="ExternalOutput").ap()

    with tile.TileContext(nc) as tc:
        with (
            tc.tile_pool(name="consts", bufs=1) as consts,
            tc.tile_pool(name="qpool", bufs=2) as qpool,
            tc.tile_pool(name="lhsp", bufs=4) as lhsp,
            tc.tile_pool(name="rhsp", bufs=4) as rhsp,
            tc.tile_pool(name="vtallp", bufs=2) as vtallp,
            tc.tile_pool(name="etp", bufs=3) as etp,
            tc.tile_pool(name="fop", bufs=2) as fop,
            tc.tile_pool(name="recp", bufs=2) as recp,
            tc.tile_pool(name="wcs", bufs=2) as wcs,
            tc.tile_pool(name="spp", bufs=2, space="PSUM") as spp,
            tc.tile_pool(name="opp", bufs=2, space="PSUM") as opp,
        ):
            relh4c = consts.tile([128, 252], BF16, tag="relh4c")
            relw4c = consts.tile([128, 252], BF16, tag="relw4c")
            nc.sync.dma_start(relh4c[:], relh4_d[:])
            nc.sync.dma_start(relw4c[:], relw4_d[:])
            # expand the 32 shifted block-diagonal slabs on-device (DVE,
            # off the ACT critical path; walrus needs 1-free-dim weights)
            relw4 = consts.tile([128, 4096], BF16, tag="relw4")
            relh4 = consts.tile([128, 4096], BF16, tag="relh4")
            relw4c_v = relw4c[:].rearrange("p (h j) -> p h j", h=4)
            relh4c_v = relh4c[:].rearrange("p (h j) -> p h j", h=4)
            for y in range(32):
                nc.vector.tensor_copy(
                    relh4[:, 128 * y : 128 * y + 128]
                        .rearrange("p (h j) -> p h j", h=4),
                    relh4c_v[:, :, 31 - y : 63 - y],
                )
            for y in range(32):
                nc.vector.tensor_copy(
                    relw4[:, 128 * y : 128 * y + 128]
                        .rearrange("p (h j) -> p h j", h=4),
                    relw4c_v[:, :, 31 - y : 63 - y],
                )

            for it in range(niters):
                qs = [qpool.tile([128, HW], BF16, tag="qs", name=f"qs{it}_{i}")
                      for i in range(2)]
                for g in range(2):
                    for half in range(2):
                        nc.sync.dma_start(
                            qs[g][:, 512 * half : 512 * (half + 1)],
                            q4_d[128 * g : 128 * (g + 1),
                                 512 * half : 512 * (half + 1)])

                # compact rel tables Wc/Hc for both 4-head groups
                # Wc stored y-major (col = 32y + x), Hc natural l-major
                wc_sb, hc_sb = [], []
                for g in range(2):
                    qv = qs[g][:].rearrange("p (x y) -> p x y", y=32)
                    wcp = spp.tile([128, HW], F32, tag="sp", name=f"wcp{it}_{g}")
                    hcp = spp.tile([128, HW], F32, tag="sp", name=f"hcp{it}_{g}")
                    for x in range(32):
                        nc.tensor.matmul(
                            out=hcp[:, 32 * x : 32 * x + 32],
                            lhsT=relh4[:, 128 * x : 128 * x + 128],
                            rhs=qs[g][:, 32 * x : 32 * x + 32],
                            start=True, stop=True,
                        )
                    for y in range(32):
                        nc.tensor.matmul(
                            out=wcp[:, 32 * y : 32 * y + 32],
                            lhsT=relw4[:, 128 * y : 128 * y + 128],
                            rhs=qv[:, :, y],
                            start=True, stop=True,
                        )
                    wt = wcs.tile([128, HW], BF16, tag="wcsb", name=f"wt{it}_{g}")
                    ht = wcs.tile([128, HW], BF16, tag="hcsb", name=f"ht{it}_{g}")
                    nc.vector.tensor_copy(
                        wt[:].rearrange("p (x y) -> p y x", y=32),
                        wcp[:].rearrange("p (y x) -> p y x", x=32),
                    )
                    nc.vector.tensor_copy(ht[:], hcp[:])
                    wc_sb.append(wt)
                    hc_sb.append(ht)

                for h in range(NH):
                    g, r = h // 4, (h % 4) * 32
                    lhs_t = lhsp.tile([96, HW], BF16, tag="lhs", name=f"lhs{it}_{h}")
                    nc.gpsimd.dma_start(lhs_t[:], lhs_d[h])
                    rhs_t = rhsp.tile([96, HW], BF16, tag="rhs", name=f"rhs{it}_{h}")
                    nc.sync.dma_start(rhs_t[0:32, :], q4_d[128 * g + r : 128 * g + r + 32, :])
                    nc.vector.tensor_copy(rhs_t[32:64, :], wc_sb[g][r : r + 32, :])
                    nc.vector.tensor_copy(rhs_t[64:96, :], hc_sb[g][r : r + 32, :])

                    vt_all = vtallp.tile([128, 384], BF16, tag="vtall",
                                         name=f"vta{it}_{h}")
                    vtv = vt_all[:].rearrange("p (m j) -> p m j", j=48)
                    nc.sync.dma_start_transpose(vtv, v_d[h])

                    op = opp.tile([33, HW], F32, tag="op", name=f"op{it}_{h}")
                    for m in range(8):
                        sp = spp.tile([128, HW], F32, tag="sp",
                                      name=f"sp{it}_{h}_{m}")
                        et = etp.tile([128, HW], BF16, tag="et",
                                      name=f"et{it}_{h}_{m}")
                        for j in range(2):
                            nc.tensor.matmul(
                                out=sp[:, 512 * j : 512 * (j + 1)],
                                lhsT=lhs_t[:, 128 * m : 128 * (m + 1)],
                                rhs=rhs_t[:, 512 * j : 512 * (j + 1)],
                                start=True, stop=True,
                            )
                        nc.scalar.activation(et[:], sp[:], AF.Exp)
                        for j in range(2):
                            nc.tensor.matmul(
                                out=op[:, 512 * j : 512 * (j + 1)],
                                lhsT=vtv[:, m, 0:33],
                                rhs=et[:, 512 * j : 512 * (j + 1)],
                                start=(m == 0), stop=(m == 7),
                            )

                    # normalize: gpsimd-broadcast denom, reciprocal, multiply
                    den = recp.tile([1, HW], F32, tag="den",
                                    name=f"den{it}_{h}")
                    nc.vector.tensor_copy(den[:], op[32:33, :])
                    denb = recp.tile([32, HW], F32, tag="denb",
                                     name=f"denb{it}_{h}")
                    nc.gpsimd.partition_broadcast(denb[:], den[:])
                    recb = recp.tile([32, HW], F32, tag="recb",
                                     name=f"recb{it}_{h}")
                    nc.vector.reciprocal(recb[:], denb[:])
                    fo = fop.tile([32, HW], F32, tag="fo", name=f"fo{it}_{h}")
                    nc.vector.tensor_mul(fo[:], op[0:32, :], recb[:])
                    nc.sync.dma_start(out_d[32 * h : 32 * h + 32, :], fo[:])

    nc.compile()
    return nc


def kernel(x: np.ndarray, rel_w: np.ndarray, rel_h: np.ndarray) -> np.ndarray:
    x = np.asarray(x, dtype=np.float32)
    B = x.shape[0]
    n_cores = 8
    assert B == n_cores and x.shape[1:] == (768, 32, 32)

    consts, sel_bf = _host_prep_consts(np.asarray(rel_w), np.asarray(rel_h))
    in_maps = []
    for b in range(n_cores):
        m = dict(consts)
        m.update(_host_prep_core(x[b], sel_bf))
        in_maps.append(m)

    if "nc" not in _CACHE:
        _CACHE["nc"] = build_nc(niters=1, num_devices=n_cores)
    nc = _CACHE["nc"]

    res = run_bass_kernel_spmd(nc, in_maps, list(range(n_cores)))
    out = np.stack([np.asarray(res.results[b]["out"]).reshape(256, 32, 32)
                    for b in range(n_cores)])
    return out.astype(np.float32)



# revision 6
# speedup vs baseline: 1.0412x; 1.0412x over previous
"""Trainium2 Bass kernel for AttnAugmentation2d (8 cores, batch-parallel).

Contract: kernel(**inputs) takes FULL inputs
  x [8, 768, 32, 32] f32, rel_w [63, 32] f32, rel_h [63, 32] f32
and returns the FULL output [8, 256, 32, 32] f32.

Sharding: data-parallel over batch — core b computes batch element b.

Per-core computation (channels-on-partitions layout, l = x*32 + y):
  S^T[m, l] = sum_d k[d,m] q[d,l] + Wc[y'(m), l] + Hc[x'(m), l]
  computed as ONE matmul with K=96: k rows plus 0/1 selector rows that
  broadcast the compact relative-position tables Wc/Hc [32, 1024].
  Wc/Hc come from block-diagonal matmuls (4 heads at once, K=128)
  against host-built kron(I4, rel_w.T) shifted tables, per y (resp. x).

  P^T = exp(S^T), per [128, 512] half-block, split between the scalar
  engine (native Exp) and DVE (Schraudolph fast-exp: int16 round of
  x*128/ln2 + magic, bit-cast to bf16; max rel err ~3.3%, common-mode
  bias cancels in the softmax normalization).  Only ACT/DVE have PSUM
  ports, so all PSUM-sourced elementwise work lives there; GPSIMD does
  SBUF->SBUF rhs assembly and the (software-DGE) output DMAs, keeping
  blocking output waits off the SP sequencer.

  out^T[d, l] = sum_m vT[m, d] P^T[m, l], two heads per pass via PE
  column tiling: head A at tile_position (0,0) (PSUM partitions 0:64),
  head B at (0,64) (partitions 64:128); the two matmuls stream
  concurrently through disjoint column groups.  lhsT per head is
  [vT | ones32] (M=64): the ones columns produce the softmax
  denominator replicated on 32 partitions, so normalization is just
  DVE reciprocal + multiply.
Matmul operands are bf16 (1 PE cycle/row); accumulation is fp32 PSUM.
"""

import numpy as np
import ml_dtypes

import concourse.bacc as bacc
import concourse.mybir as mybir
from concourse import tile
from concourse.bass_utils import run_bass_kernel_spmd

F32 = mybir.dt.float32
BF16 = mybir.dt.bfloat16
I16 = mybir.dt.int16
AF = mybir.ActivationFunctionType
ALU = mybir.AluOpType
BF = np.dtype(ml_dtypes.bfloat16)

NH = 8
HW = 1024
SCALE = 32.0 ** -0.5

# Schraudolph fast-exp in bf16 bit space: bf16_bits(e^x) ~= x*128/ln2 + magic
EXP_A = 128.0 / float(np.log(2.0))
EXP_B = 127.0 * 128.0 - 5.6

# fraction of exp half-blocks routed to DVE (rest on scalar engine)
DVE_EXP_FRAC = 0.36

_CACHE = {}


def _host_prep_consts(rel_w: np.ndarray, rel_h: np.ndarray):
    # 32 shifted block-diagonal weight tables, one contiguous [128,128]
    # slab per y (walrus requires 1 free dim on matmul weight APs)
    i4 = np.eye(4, dtype=np.float32)
    rwT = np.asarray(rel_w, dtype=np.float32).T
    rhT = np.asarray(rel_h, dtype=np.float32).T
    relw4 = np.kron(i4, rwT)  # [128, 252]
    relh4 = np.kron(i4, rhT)
    m = np.arange(HW)
    wsel = (m[None, :] % 32 == np.arange(32)[:, None]).astype(np.float32)
    hsel = (m[None, :] // 32 == np.arange(32)[:, None]).astype(np.float32)
    sel = np.concatenate([wsel, hsel], axis=0)  # [64, 1024]
    consts = dict(
        relw4=relw4.astype(BF),
        relh4=relh4.astype(BF),
    )
    return consts, sel.astype(BF)


def _host_prep_core(x_b: np.ndarray, sel_bf: np.ndarray):
    xf = np.ascontiguousarray(np.asarray(x_b, np.float32).reshape(768, HW))
    q4 = (xf[0:256] * SCALE).astype(BF)  # scaled in fp32, rounded once
    lhs = np.empty((NH, 96, HW), dtype=BF)
    for h in range(NH):
        lhs[h, 0:32] = xf[256 + 32 * h : 288 + 32 * h].astype(BF)
        lhs[h, 32:96] = sel_bf
    # v pairs: [vT | ones] columns for two heads per slab (transposed on
    # the way to SBUF); ones columns give the softmax denominator
    # replicated over 32 partitions.
    vp = np.zeros((NH // 2, 128, HW), dtype=BF)
    for p in range(NH // 2):
        vp[p, 0:32] = xf[512 + 64 * p : 544 + 64 * p].astype(BF)
        vp[p, 32:64] = 1.0
        vp[p, 64:96] = xf[544 + 64 * p : 576 + 64 * p].astype(BF)
        vp[p, 96:128] = 1.0
    return dict(q4=q4, lhs=lhs, vp=vp)


def build_nc(niters: int = 1, num_devices: int = 8, hw_loop: bool = False):
    nc = bacc.Bacc(None, target_bir_lowering=False, debug=False,
                   num_devices=num_devices)

    q4_d = nc.dram_tensor("q4", [256, HW], BF16, kind="ExternalInput").ap()
    lhs_d = nc.dram_tensor("lhs", [NH, 96, HW], BF16, kind="ExternalInput").ap()
    vp_d = nc.dram_tensor("vp", [NH // 2, 128, HW], BF16,
                          kind="ExternalInput").ap()
    relw4_d = nc.dram_tensor("relw4", [128, 252], BF16, kind="ExternalInput").ap()
    relh4_d = nc.dram_tensor("relh4", [128, 252], BF16, kind="ExternalInput").ap()
    out_d = nc.dram_tensor("out", [256, HW], F32, kind="ExternalOutput").ap()

    with tile.TileContext(nc) as tc:
        with (
            tc.tile_pool(name="consts", bufs=1) as consts,
            tc.tile_pool(name="qpool", bufs=3) as qpool,
            tc.tile_pool(name="wtp", bufs=2) as wtp,
            tc.tile_pool(name="lhsp", bufs=6) as lhsp,
            tc.tile_pool(name="rhsp", bufs=8) as rhsp,
            tc.tile_pool(name="vtp", bufs=4) as vtp,
            tc.tile_pool(name="etp", bufs=8) as etp,
            tc.tile_pool(name="fop", bufs=3) as fop,
            tc.tile_pool(name="recp", bufs=4) as recp,
            tc.tile_pool(name="sppA", bufs=2, space="PSUM") as sppA,
            tc.tile_pool(name="sppB", bufs=2, space="PSUM") as sppB,
            tc.tile_pool(name="opp", bufs=2, space="PSUM") as opp,
        ):
            relh4c = consts.tile([128, 252], BF16, tag="relh4c")
            relw4c = consts.tile([128, 252], BF16, tag="relw4c")
            nc.sync.dma_start(relh4c[:], relh4_d[:])
            nc.sync.dma_start(relw4c[:], relw4_d[:])
            # expand the 32 shifted block-diagonal slabs on-device (DVE,
            # prologue only; walrus needs 1-free-dim weights)
            relw4 = consts.tile([128, 4096], BF16, tag="relw4")
            relh4 = consts.tile([128, 4096], BF16, tag="relh4")
            relw4c_v = relw4c[:].rearrange("p (h j) -> p h j", h=4)
            relh4c_v = relh4c[:].rearrange("p (h j) -> p h j", h=4)
            for y in range(32):
                nc.vector.tensor_copy(
                    relh4[:, 128 * y : 128 * y + 128]
                        .rearrange("p (h j) -> p h j", h=4),
                    relh4c_v[:, :, 31 - y : 63 - y],
                )
            for y in range(32):
                nc.vector.tensor_copy(
                    relw4[:, 128 * y : 128 * y + 128]
                        .rearrange("p (h j) -> p h j", h=4),
                    relw4c_v[:, :, 31 - y : 63 - y],
                )

            # deterministic weighted round-robin for the exp engine split
            state = {"acc": 0.0}

            def exp_half(et_half, sp_half):
                state["acc"] += DVE_EXP_FRAC
                if state["acc"] >= 1.0:
                    state["acc"] -= 1.0
                    nc.vector.tensor_scalar(
                        et_half.bitcast(I16), sp_half,
                        EXP_A, EXP_B, ALU.mult, ALU.add)
                else:
                    nc.scalar.activation(et_half, sp_half, AF.Exp)

            def body(it):
                # ---- per-group: q load, Wc/Hc matmuls, rhs assembly ----
                rhs_ts = []
                for g in range(2):
                    qs = qpool.tile([128, HW], BF16, tag="qs",
                                    name=f"qs{it}_{g}")
                    for half in range(2):
                        nc.sync.dma_start(
                            qs[:, 512 * half : 512 * (half + 1)],
                            q4_d[128 * g : 128 * (g + 1),
                                 512 * half : 512 * (half + 1)])
                    qv = qs[:].rearrange("p (x y) -> p x y", y=32)
                    wcp = opp.tile([128, HW], F32, tag="op",
                                   name=f"wcp{it}_{g}")
                    hcp = opp.tile([128, HW], F32, tag="op",
                                   name=f"hcp{it}_{g}")
                    for x in range(32):
                        nc.tensor.matmul(
                            out=hcp[:, 32 * x : 32 * x + 32],
                            lhsT=relh4[:, 128 * x : 128 * x + 128],
                            rhs=qs[:, 32 * x : 32 * x + 32],
                            start=True, stop=True,
                        )
                    for y in range(32):
                        nc.tensor.matmul(
                            out=wcp[:, 32 * y : 32 * y + 32],
                            lhsT=relw4[:, 128 * y : 128 * y + 128],
                            rhs=qv[:, :, y],
                            start=True, stop=True,
                        )
                    # big PSUM->SBUF copies: W transposed to l-major (DVE),
                    # H straight (scalar engine)
                    wt = wtp.tile([128, HW], BF16, tag="wt",
                                  name=f"wt{it}_{g}")
                    ht = wtp.tile([128, HW], BF16, tag="ht",
                                  name=f"ht{it}_{g}")
                    nc.vector.tensor_copy(
                        wt[:].rearrange("p (x y) -> p y x", y=32),
                        wcp[:].rearrange("p (y x) -> p y x", x=32),
                    )
                    nc.scalar.activation(ht[:], hcp[:], AF.Copy)
                    # per-head rhs assembly: [q ; Wc ; Hc] on 96 partitions
                    # (SBUF->SBUF partition shifts on gpsimd)
                    for hh in range(4):
                        h = 4 * g + hh
                        r = 32 * hh
                        rhs_t = rhsp.tile([96, HW], BF16, tag="rhs",
                                          name=f"rhs{it}_{h}")
                        nc.sync.dma_start(rhs_t[0:32, :],
                                          q4_d[32 * h : 32 * h + 32, :])
                        nc.gpsimd.tensor_copy(rhs_t[32:64, :], wt[r : r + 32, :])
                        nc.gpsimd.tensor_copy(rhs_t[64:96, :], ht[r : r + 32, :])
                        rhs_ts.append(rhs_t)

                # ---- per pair of heads: S, exp, col-tiled AV, normalize ----
                pending_out = []
                for p in range(4):
                    lhsA = lhsp.tile([96, HW], BF16, tag="lhs",
                                     name=f"lhsA{it}_{p}")
                    nc.sync.dma_start(lhsA[:], lhs_d[2 * p])
                    lhsB = lhsp.tile([96, HW], BF16, tag="lhs",
                                     name=f"lhsB{it}_{p}")
                    nc.sync.dma_start(lhsB[:], lhs_d[2 * p + 1])
                    vt = vtp.tile([128, HW], BF16, tag="vt",
                                  name=f"vt{it}_{p}")
                    vtv = vt[:].rearrange("p (m j) -> p m j", j=128)
                    nc.sync.dma_start_transpose(vtv, vp_d[p])

                    op = opp.tile([128, HW], F32, tag="op", name=f"op{it}_{p}")
                    ets = {}

                    def emit_av(mb):
                        for j in range(2):
                            nc.tensor.matmul(
                                out=op[0:64, 512 * j : 512 * (j + 1)],
                                lhsT=vtv[:, mb, 0:64],
                                rhs=ets[(mb, 0)][:, 512 * j : 512 * (j + 1)],
                                start=(mb == 0), stop=(mb == 7),
                            )
                            nc.tensor.matmul(
                                out=op[64:128, 512 * j : 512 * (j + 1)],
                                lhsT=vtv[:, mb, 64:128],
                                rhs=ets[(mb, 1)][:, 512 * j : 512 * (j + 1)],
                                start=(mb == 0), stop=(mb == 7),
                            )

                    for mb in range(8):
                        for side, (lhs_t, rhs_t, pool) in enumerate(
                            [(lhsA, rhs_ts[2 * p], sppA),
                             (lhsB, rhs_ts[2 * p + 1], sppB)]):
                            et = etp.tile([128, HW], BF16, tag="et",
                                          name=f"et{it}_{p}_{mb}_{side}")
                            for j in range(2):
                                sp = pool.tile([128, 512], F32, tag="sp",
                                               name=f"sp{it}_{p}_{mb}_{side}_{j}")
                                nc.tensor.matmul(
                                    out=sp[:],
                                    lhsT=lhs_t[:, 128 * mb : 128 * (mb + 1)],
                                    rhs=rhs_t[:, 512 * j : 512 * (j + 1)],
                                    start=True, stop=True,
                                )
                                exp_half(et[:, 512 * j : 512 * (j + 1)], sp[:])
                            ets[(mb, side)] = et
                        if mb > 0:
                            emit_av(mb - 1)
                    emit_av(7)

                    # normalize both heads: rec = 1/den, fo = num * rec
                    fo = fop.tile([64, HW], F32, tag="fo", name=f"fo{it}_{p}")
                    for side in range(2):
                        base = 64 * side
                        rec = recp.tile([32, HW], F32, tag="rec",
                                        name=f"rec{it}_{p}_{side}")
                        nc.vector.reciprocal(rec[:], op[base + 32 : base + 64, :])
                        nc.vector.tensor_mul(fo[32 * side : 32 * side + 32, :],
                                             op[base : base + 32, :], rec[:])
                    # defer the output DMA one pair so its wait is satisfied
                    # at dispatch (keeps Pool.SEQ from head-of-line blocking)
                    for pp, pfo in pending_out:
                        nc.gpsimd.dma_start(out_d[64 * pp : 64 * pp + 64, :],
                                            pfo[:])
                    pending_out = [(p, fo)]
                for pp, pfo in pending_out:
                    nc.gpsimd.dma_start(out_d[64 * pp : 64 * pp + 64, :], pfo[:])

            if hw_loop and niters > 1:
                with tc.For_i(0, niters, 1,
                              hint_engines=tuple(mybir.ALL_ENGINES)):
                    body(0)
            else:
                for it in range(niters):
                    body(it)

    nc.compile()
    return nc


def kernel(x: np.ndarray, rel_w: np.ndarray, rel_h: np.ndarray) -> np.ndarray:
    x = np.asarray(x, dtype=np.float32)
    B = x.shape[0]
    n_cores = 8
    assert B == n_cores and x.shape[1:] == (768, 32, 32)

    consts, sel_bf = _host_prep_consts(np.asarray(rel_w), np.asarray(rel_h))
    in_maps = []
    for b in range(n_cores):
        m = dict(consts)
        m.update(_host_prep_core(x[b], sel_bf))
        in_maps.append(m)

    if "nc" not in _CACHE:
        _CACHE["nc"] = build_nc(niters=1, num_devices=n_cores)
    nc = _CACHE["nc"]

    res = run_bass_kernel_spmd(nc, in_maps, list(range(n_cores)))
    out = np.stack([np.asarray(res.results[b]["out"]).reshape(256, 32, 32)
                    for b in range(n_cores)])
    return out.astype(np.float32)


# revision 19
# speedup vs baseline: 1.4177x; 1.3616x over previous
"""Trainium2 Bass kernel for AttnAugmentation2d (8 cores, batch-parallel).

Contract: kernel(**inputs) takes FULL inputs
  x [8, 768, 32, 32] f32, rel_w [63, 32] f32, rel_h [63, 32] f32
and returns the FULL output [8, 256, 32, 32] f32.

Sharding: data-parallel over batch — core b computes batch element b.

Per-core computation (channels-on-partitions layout, l = x*32 + y):
  S^T[m, l] = sum_d k[d,m] q[d,l] + Wc[y'(m), l] + Hc[x'(m), l]
  computed as ONE matmul with K=96: k rows plus 0/1 selector rows that
  broadcast the compact relative-position tables Wc/Hc [32, 1024].
  Wc/Hc come from block-diagonal matmuls (4 heads at once, K=128)
  against host-built kron(I4, rel_w.T) shifted tables, per y (resp. x).

  P^T = exp(S^T), per [128, 512] half-block, split between the scalar
  engine (native Exp) and DVE (Schraudolph fast-exp: int16 round of
  x*128/ln2 + magic, bit-cast to bf16; max rel err ~3.3%, common-mode
  bias cancels in the softmax normalization).  Only ACT/DVE have PSUM
  ports, so all PSUM-sourced elementwise work lives there; GPSIMD does
  SBUF->SBUF rhs assembly and the (software-DGE) output DMAs, keeping
  blocking output waits off the SP sequencer.

  out^T[d, l] = sum_m vT[m, d] P^T[m, l], two heads per pass via PE
  column tiling: head A at tile_position (0,0) (PSUM partitions 0:64),
  head B at (0,64) (partitions 64:128); the two matmuls stream
  concurrently through disjoint column groups.  lhsT per head is
  [vT | ones32] (M=64): the ones columns produce the softmax
  denominator replicated on 32 partitions, so normalization is just
  DVE reciprocal + multiply.
Matmul operands are bf16 (1 PE cycle/row); accumulation is fp32 PSUM.
"""

import numpy as np
import ml_dtypes

import concourse.bacc as bacc
import concourse.mybir as mybir
from concourse import tile
from concourse.bass_utils import run_bass_kernel_spmd

F32 = mybir.dt.float32
BF16 = mybir.dt.bfloat16
I16 = mybir.dt.int16
AF = mybir.ActivationFunctionType
ALU = mybir.AluOpType
BF = np.dtype(ml_dtypes.bfloat16)

NH = 8
HW = 1024
SCALE = 32.0 ** -0.5

# Schraudolph fast-exp in bf16 bit space: bf16_bits(e^x) ~= x*128/ln2 + magic
EXP_A = 128.0 / float(np.log(2.0))
EXP_B = 127.0 * 128.0 - 5.6

# fraction of exp blocks routed to DVE (rest on scalar engine)
DVE_EXP_FRAC = 0.40

_CACHE = {}


def _host_prep_consts(rel_w: np.ndarray, rel_h: np.ndarray):
    # 32 shifted block-diagonal weight tables, one contiguous [128,128]
    # slab per y (walrus requires 1 free dim on matmul weight APs)
    i4 = np.eye(4, dtype=np.float32)
    rwT = np.asarray(rel_w, dtype=np.float32).T
    rhT = np.asarray(rel_h, dtype=np.float32).T
    relw4 = np.kron(i4, rwT)  # [128, 252]
    relh4 = np.kron(i4, rhT)
    m = np.arange(HW)
    wsel = (m[None, :] % 32 == np.arange(32)[:, None]).astype(np.float32)
    hsel = (m[None, :] // 32 == np.arange(32)[:, None]).astype(np.float32)
    sel = np.concatenate([wsel, hsel], axis=0)  # [64, 1024]
    consts = dict(
        relw4=relw4.astype(BF),
        relh4=relh4.astype(BF),
    )
    return consts, sel.astype(BF)


def _host_prep_core(x_b: np.ndarray, sel_bf: np.ndarray):
    xf = np.ascontiguousarray(np.asarray(x_b, np.float32).reshape(768, HW))
    q4 = (xf[0:256] * SCALE).astype(BF)  # scaled in fp32, rounded once
    lhs = np.empty((NH, 96, HW), dtype=BF)
    for h in range(NH):
        lhs[h, 0:32] = xf[256 + 32 * h : 288 + 32 * h].astype(BF)
        lhs[h, 32:96] = sel_bf
    # v pairs: [vT | ones] columns for two heads per slab (transposed on
    # the way to SBUF); ones columns give the softmax denominator
    # replicated over 32 partitions.
    vp = np.zeros((NH // 2, 128, HW), dtype=BF)
    for p in range(NH // 2):
        vp[p, 0:32] = xf[512 + 64 * p : 544 + 64 * p].astype(BF)
        vp[p, 32:64] = 1.0
        vp[p, 64:96] = xf[544 + 64 * p : 576 + 64 * p].astype(BF)
        vp[p, 96:128] = 1.0
    return dict(q4=q4, lhs=lhs, vp=vp)


def build_nc(niters: int = 1, num_devices: int = 8, hw_loop: bool = False):
    nc = bacc.Bacc(None, target_bir_lowering=False, debug=False,
                   num_devices=num_devices)

    q4_d = nc.dram_tensor("q4", [256, HW], BF16, kind="ExternalInput").ap()
    lhs_d = nc.dram_tensor("lhs", [NH, 96, HW], BF16, kind="ExternalInput").ap()
    vp_d = nc.dram_tensor("vp", [NH // 2, 128, HW], BF16,
                          kind="ExternalInput").ap()
    relw4_d = nc.dram_tensor("relw4", [128, 252], BF16, kind="ExternalInput").ap()
    relh4_d = nc.dram_tensor("relh4", [128, 252], BF16, kind="ExternalInput").ap()
    out_d = nc.dram_tensor("out", [256, HW], F32, kind="ExternalOutput").ap()

    with tile.TileContext(nc) as tc:
        with (
            tc.tile_pool(name="consts", bufs=1) as consts,
            tc.tile_pool(name="qpool", bufs=3) as qpool,
            tc.tile_pool(name="lhsp", bufs=6) as lhsp,
            tc.tile_pool(name="rhsp", bufs=8) as rhsp,
            tc.tile_pool(name="vtp", bufs=4) as vtp,
            tc.tile_pool(name="etp", bufs=8) as etp,
            tc.tile_pool(name="fop", bufs=3) as fop,
            tc.tile_pool(name="recp", bufs=4) as recp,
            tc.tile_pool(name="spp", bufs=2, space="PSUM") as spp,
            tc.tile_pool(name="opp", bufs=2, space="PSUM") as opp,
        ):
            relh4c = consts.tile([128, 252], BF16, tag="relh4c")
            relw4c = consts.tile([128, 252], BF16, tag="relw4c")
            nc.sync.dma_start(relh4c[:], relh4_d[:])
            nc.sync.dma_start(relw4c[:], relw4_d[:])
            # expand the 32 shifted block-diagonal slabs on-device (DVE,
            # prologue only; walrus needs 1-free-dim weights)
            relw4 = consts.tile([128, 4096], BF16, tag="relw4")
            relh4 = consts.tile([128, 4096], BF16, tag="relh4")
            relw4c_v = relw4c[:].rearrange("p (h j) -> p h j", h=4)
            relh4c_v = relh4c[:].rearrange("p (h j) -> p h j", h=4)
            for y in range(32):
                nc.vector.tensor_copy(
                    relh4[:, 128 * y : 128 * y + 128]
                        .rearrange("p (h j) -> p h j", h=4),
                    relh4c_v[:, :, 31 - y : 63 - y],
                )
            for y in range(32):
                nc.vector.tensor_copy(
                    relw4[:, 128 * y : 128 * y + 128]
                        .rearrange("p (h j) -> p h j", h=4),
                    relw4c_v[:, :, 31 - y : 63 - y],
                )

            # deterministic weighted round-robin for the exp engine split
            state = {"acc": 0.0}

            def exp_half(et_half, sp_half):
                state["acc"] += DVE_EXP_FRAC
                if state["acc"] >= 1.0:
                    state["acc"] -= 1.0
                    nc.vector.tensor_scalar(
                        et_half.bitcast(I16), sp_half,
                        EXP_A, EXP_B, ALU.mult, ALU.add)
                else:
                    nc.scalar.activation(et_half, sp_half, AF.Exp)

            def body(it):
                # ---- per-group: q load, Wc/Hc matmuls, rhs assembly ----
                rhs_ts = []
                for g in range(2):
                    qs = qpool.tile([128, HW], BF16, tag="qs",
                                    name=f"qs{it}_{g}")
                    for half in range(2):
                        nc.sync.dma_start(
                            qs[:, 512 * half : 512 * (half + 1)],
                            q4_d[128 * g : 128 * (g + 1),
                                 512 * half : 512 * (half + 1)])
                    qv = qs[:].rearrange("p (x y) -> p x y", y=32)
                    wcp = opp.tile([128, HW], F32, tag="op",
                                   name=f"wcp{it}_{g}")
                    hcp = opp.tile([128, HW], F32, tag="op",
                                   name=f"hcp{it}_{g}")
                    for x in range(32):
                        nc.tensor.matmul(
                            out=hcp[:, 32 * x : 32 * x + 32],
                            lhsT=relh4[:, 128 * x : 128 * x + 128],
                            rhs=qs[:, 32 * x : 32 * x + 32],
                            start=True, stop=True,
                        )
                    # W matmuls write l-major via strided out APs (16 cols
                    # stride 32 stays within one PSUM bank per half) so the
                    # per-head copy below is contiguous — the strided fix-up
                    # copy costs ~4.1us on HW, the strided mm out is free
                    wcpv = wcp[:].rearrange("p (x y) -> p x y", y=32)
                    for y in range(32):
                        for xh in range(2):
                            nc.tensor.matmul(
                                out=wcpv[:, 16 * xh : 16 * xh + 16, y],
                                lhsT=relw4[:, 128 * y : 128 * y + 128],
                                rhs=qv[:, 16 * xh : 16 * xh + 16, y],
                                start=True, stop=True,
                            )
                    # per-head rhs assembly: [q ; Wc ; Hc] on 96 partitions,
                    # direct PSUM->SBUF slices (gpsimd has no PSUM port and
                    # its SBUF copies are ~3.2us on HW — avoid entirely):
                    # W slice transposed to l-major on DVE, H slice on ACT.
                    for hh in range(4):
                        h = 4 * g + hh
                        r = 32 * hh
                        rhs_t = rhsp.tile([96, HW], BF16, tag="rhs",
                                          name=f"rhs{it}_{h}")
                        nc.sync.dma_start(rhs_t[0:32, :],
                                          q4_d[32 * h : 32 * h + 32, :])
                        nc.vector.tensor_copy(rhs_t[32:64, :],
                                              wcp[r : r + 32, :])
                        nc.scalar.activation(rhs_t[64:96, :],
                                             hcp[r : r + 32, :], AF.Copy)
                        rhs_ts.append(rhs_t)

                # ---- per pair of heads: S, exp, col-tiled AV, normalize ----
                pending_out = []
                for p in range(4):
                    lhsA = lhsp.tile([96, HW], BF16, tag="lhs",
                                     name=f"lhsA{it}_{p}")
                    nc.sync.dma_start(lhsA[:], lhs_d[2 * p])
                    lhsB = lhsp.tile([96, HW], BF16, tag="lhs",
                                     name=f"lhsB{it}_{p}")
                    nc.sync.dma_start(lhsB[:], lhs_d[2 * p + 1])
                    vt = vtp.tile([128, HW], BF16, tag="vt",
                                  name=f"vt{it}_{p}")
                    vtv = vt[:].rearrange("p (m j) -> p m j", j=128)
                    nc.sync.dma_start_transpose(vtv, vp_d[p])

                    op = opp.tile([128, HW], F32, tag="op", name=f"op{it}_{p}")
                    ets = {}

                    def emit_av(mb):
                        # adjacent matmuls use disjoint col groups so they
                        # stream concurrently through separate XBUSes
                        for j in range(2):
                            nc.tensor.matmul(
                                out=op[0:64, 512 * j : 512 * (j + 1)],
                                lhsT=vtv[:, mb, 0:64],
                                rhs=ets[(mb, 0)][:, 512 * j : 512 * (j + 1)],
                                start=(mb == 0), stop=(mb == 7),
                            )
                            nc.tensor.matmul(
                                out=op[64:128, 512 * j : 512 * (j + 1)],
                                lhsT=vtv[:, mb, 64:128],
                                rhs=ets[(mb, 1)][:, 512 * j : 512 * (j + 1)],
                                start=(mb == 0), stop=(mb == 7),
                            )

                    for mb in range(8):
                        for side, (lhs_t, rhs_t) in enumerate(
                            [(lhsA, rhs_ts[2 * p]),
                             (lhsB, rhs_ts[2 * p + 1])]):
                            et = etp.tile([128, HW], BF16, tag="et",
                                          name=f"et{it}_{p}_{mb}_{side}")
                            sp = spp.tile([128, HW], F32, tag="sp",
                                          name=f"sp{it}_{p}_{mb}_{side}")
                            for j in range(2):
                                nc.tensor.matmul(
                                    out=sp[:, 512 * j : 512 * (j + 1)],
                                    lhsT=lhs_t[:, 128 * mb : 128 * (mb + 1)],
                                    rhs=rhs_t[:, 512 * j : 512 * (j + 1)],
                                    start=True, stop=True,
                                )
                            exp_half(et[:], sp[:])
                            ets[(mb, side)] = et
                        if mb > 0:
                            emit_av(mb - 1)
                    emit_av(7)

                    # normalize both heads: rec = 1/den, fo = num * rec
                    fo = fop.tile([64, HW], F32, tag="fo", name=f"fo{it}_{p}")
                    for side in range(2):
                        base = 64 * side
                        den = recp.tile([32, HW], F32, tag="den",
                                        name=f"den{it}_{p}_{side}")
                        nc.scalar.activation(den[:], op[base + 32 : base + 64, :],
                                             AF.Copy)
                        rec = recp.tile([32, HW], F32, tag="rec",
                                        name=f"rec{it}_{p}_{side}")
                        # custom-DVE ucode is SBUF-only, hence the ACT bounce
                        nc.vector.reciprocal_approx_fast(out=rec[:], in_=den[:])
                        nc.vector.tensor_mul(fo[32 * side : 32 * side + 32, :],
                                             op[base : base + 32, :], rec[:])
                    # defer the output DMA one pair so its wait is satisfied
                    # at dispatch (keeps Pool.SEQ from head-of-line blocking)
                    for pp, pfo in pending_out:
                        nc.gpsimd.dma_start(out_d[64 * pp : 64 * pp + 64, :],
                                            pfo[:])
                    pending_out = [(p, fo)]
                for pp, pfo in pending_out:
                    nc.gpsimd.dma_start(out_d[64 * pp : 64 * pp + 64, :], pfo[:])

            if hw_loop and niters > 1:
                with tc.For_i(0, niters, 1,
                              hint_engines=tuple(mybir.ALL_ENGINES)):
                    body(0)
            else:
                for it in range(niters):
                    body(it)

    nc.compile()
    return nc


def kernel(x: np.ndarray, rel_w: np.ndarray, rel_h: np.ndarray) -> np.ndarray:
    x = np.asarray(x, dtype=np.float32)
    B = x.shape[0]
    n_cores = 8
    assert B == n_cores and x.shape[1:] == (768, 32, 32)

    consts, sel_bf = _host_prep_consts(np.asarray(rel_w), np.asarray(rel_h))
    in_maps = []
    for b in range(n_cores):
        m = dict(consts)
        m.update(_host_prep_core(x[b], sel_bf))
        in_maps.append(m)

    if "nc" not in _CACHE:
        _CACHE["nc"] = build_nc(niters=1, num_devices=n_cores)
    nc = _CACHE["nc"]

    res = run_bass_kernel_spmd(nc, in_maps, list(range(n_cores)))
    out = np.stack([np.asarray(res.results[b]["out"]).reshape(256, 32, 32)
                    for b in range(n_cores)])
    return out.astype(np.float32)
